# revision 9
# baseline (speedup 1.0000x reference)
"""Trainium2 Bass kernel for chunked sliding-window + global-prefix causal self-attention.

Module: x[B,S,D] -> qkv proj -> windowed attention (W=512 window, 16 global
prefix keys) -> out proj.  B=2, S=4096, D=1024, H=16, Dh=64.

Sharding (8 cores): sequence-parallel over query chunk pairs.
  core i: batch b = i//4, query chunks (2j, 2j+1), j = i%4  -> 1024 query rows.
  Each core re-projects the K/V rows its chunks attend to (3 chunks of 512 =
  1536 kv tokens, front chunk zero-padded for j=0) plus the 16 prefix tokens.
  Output is row-sharded -> gather is pure concatenation.

On-chip layout is feature-major (xT = x.T fed from host), so no transposes
are ever needed:
  qT/kT  [f, tok]   (w stationary, xT moving)
  V      [tok, f]   (xT stationary, w moving), augmented with a ones column
                    per head so the attention-value matmul also produces
                    softmax row-sums for free.
  scores are computed transposed S.T [key, query]; the score matmul moving
  range is restricted to the causally-valid query window per key block.
  Row-sums for all 16 heads of a chunk are DMA-collected into one [16,W]
  tile so the softmax reciprocal is a single full-width DVE op; the
  normalization multiply is applied in place on the o.T tiles before the
  output projection.
"""

import os
import sys

import numpy as np

for _p in ("/opt/trn_rl_repo", os.path.expanduser("~/trn_rl_repo")):
    if os.path.isdir(_p) and _p not in sys.path:
        sys.path.insert(0, _p)

import ml_dtypes  # noqa: E402

import concourse.bass as bass  # noqa: E402
import concourse.mybir as mybir  # noqa: E402
import concourse.tile as tile  # noqa: E402
from concourse import bacc  # noqa: E402
from concourse.bass_utils import run_bass_kernel_spmd  # noqa: E402

F32 = mybir.dt.float32
F32R = mybir.dt.float32r
BF16 = mybir.dt.bfloat16

# problem constants (hardcoded per contest contract)
B, S, D, H, DH = 2, 4096, 1024, 16, 64
W, PRE = 512, 16
P = 128
NCH = 2                  # query chunks per core
TOK = NCH * W            # 1024 query tokens per core
KTOK = TOK + W           # 1536 kv tokens per core
KTOKP = KTOK + PRE       # + prefix tokens appended as cols [1536:1552]
FB = D // P              # 8 feature blocks of 128
NKB = 8                  # tail key blocks (128 keys) per chunk
NVT = KTOK // P          # 12 tail token tiles; tile 12 = prefix tokens
SCALE = 1.0 / 8.0        # 1/sqrt(DH)
NEG = -10000.0           # additive mask value (pre-scale)
BIAS_NEG = -50.0         # ACT bias mask value (post-scale)
N_CORES = 8

MODE = os.environ.get("KERNEL_MODE", "bf16")  # bf16 | f32r | f32


def build_nc(mode=MODE):
    nc = bacc.Bacc("TRN2", target_bir_lowering=False, debug=False)
    md = {"bf16": BF16, "f32r": F32R, "f32": F32}[mode]
    is4 = md != BF16  # 4-byte storage dtypes share fp32 bit layout

    xT = nc.dram_tensor("xT", [D, KTOKP], md, kind="ExternalInput").ap()
    wqkvT = nc.dram_tensor("wqkvT", [D, 3 * D], md, kind="ExternalInput").ap()
    woutT = nc.dram_tensor("woutT", [D, D], md, kind="ExternalInput").ap()
    trim = nc.dram_tensor("trim", [P, 2 * P], md, kind="ExternalInput").ap()
    kbias = nc.dram_tensor("kbias", [P, NCH * 9], F32, kind="ExternalInput").ap()
    premask = nc.dram_tensor("premask", [PRE, W], md, kind="ExternalInput").ap()
    out = nc.dram_tensor("out", [TOK, D], F32, kind="ExternalOutput").ap()

    from contextlib import ExitStack

    def msets(t, val):
        nc.gpsimd.memset(t.bitcast(F32) if is4 else t, val)

    lp = nc.allow_low_precision(reason="bf16/fp32r matmul chain within 2e-2 rel tol")
    lp.__enter__()
    with tile.TileContext(nc) as tc:
        with (
            tc.tile_pool(name="pp", bufs=1) as pp,                    # persistent tiles
            tc.tile_pool(name="ps2", bufs=2, space="PSUM") as ps2,    # proj / av
            tc.tile_pool(name="ps3", bufs=3, space="PSUM") as ps3,    # scores
            tc.tile_pool(name="ps1", bufs=1, space="PSUM") as ps1,    # bcast
        ):
            phaseA = ExitStack()
            px = phaseA.enter_context(tc.tile_pool(name="px", bufs=1))
            pwqk = phaseA.enter_context(tc.tile_pool(name="pwqk", bufs=16))
            pwv = phaseA.enter_context(tc.tile_pool(name="pwv", bufs=8))
            # ---- constants / masks ----
            tri_t = pp.tile([P, 2 * P], md, tag="tri")
            nc.sync.dma_start(tri_t[:], trim)
            kb_t = pp.tile([P, NCH * 9], F32, tag="kbias")
            nc.sync.dma_start(kb_t[:], kbias)
            pm_t = pp.tile([PRE, W], md, tag="premask")
            nc.sync.dma_start(pm_t[:], premask)
            ones_t = pp.tile([P, 64], md, tag="ones")
            msets(ones_t[:], 1.0)

            # ---- load xT (feature-major activations) ----
            xt = []
            for i in range(FB):
                t = px.tile([P, KTOKP], md, tag=f"xt{i}")
                nc.sync.dma_start(t[:], xT[P * i : P * (i + 1), :])
                xt.append(t)

            # ---- V projection: V[tok, f] (+ per-head ones column) ----
            # vaug[tb] cols: head h -> [65h : 65h+64] = V, col 65h+64 = 1.0
            vaug = []
            for tb in range(NVT + 1):
                t = pp.tile([P, H * 65], md, tag=f"va{tb}")
                msets(t.rearrange("p (h e) -> p h e", e=65)[:, :, 64:65], 1.0)
                vaug.append(t)
            for vh in range(2):  # 512-wide halves of the 1024 v features
                wv = []
                for db in range(FB):
                    t = pwv.tile([P, W], md)
                    nc.sync.dma_start(
                        t[:], wqkvT[P * db : P * (db + 1), 2 * D + W * vh : 2 * D + W * (vh + 1)]
                    )
                    wv.append(t)
                for tb in range(NVT + 1):
                    ps = ps2.tile([P, W], F32, tag="pa")
                    ncols = P if tb < NVT else PRE
                    for db in range(FB):
                        nc.tensor.matmul(
                            ps[:ncols, :],
                            xt[db][:, P * tb : P * tb + ncols],
                            wv[db][:],
                            start=(db == 0),
                            stop=(db == FB - 1),
                        )
                    # scatter into vaug stride-65 head slots (scalar engine: idle here)
                    nc.scalar.copy(
                        vaug[tb].rearrange("p (h e) -> p h e", e=65)[
                            :ncols, 8 * vh : 8 * (vh + 1), 0:64
                        ],
                        ps.rearrange("p (h e) -> p h e", e=64)[:ncols, 0:8, :],
                    )

            # ---- Q/K projections: qT/kT [f, tok] ----
            # whole-row weight slabs (contiguous 2KB DMA lines), V-proj style
            # uninterrupted 8-matmul accumulation chains per output tile.
            qt = [pp.tile([P, TOK], md, tag=f"qt{i}", name=f"qt{i}") for i in range(FB)]
            kt = [pp.tile([P, KTOKP], md, tag=f"kt{i}", name=f"kt{i}") for i in range(FB)]
            wq = []
            wk = []
            for db in range(FB):
                t = pwqk.tile([P, D], md)
                nc.sync.dma_start(t[:], wqkvT[P * db : P * (db + 1), 0:D])
                wq.append(t)
            for db in range(FB):
                t = pwqk.tile([P, D], md)
                nc.sync.dma_start(t[:], wqkvT[P * db : P * (db + 1), D : 2 * D])
                wk.append(t)
            copy_engines = [nc.vector.tensor_copy, nc.scalar.copy]
            cpi = 0
            for fb in range(FB):
                # q token halves {0,1} (query tokens = xT cols [512:1536))
                for th in range(2):
                    ps = ps2.tile([P, W], F32, tag="pa")
                    for db in range(FB):
                        nc.tensor.matmul(
                            ps,
                            wq[db][:, P * fb : P * (fb + 1)],
                            xt[db][:, W + W * th : W + W * (th + 1)],
                            start=(db == 0),
                            stop=(db == FB - 1),
                        )
                    copy_engines[cpi % 2](qt[fb][:, W * th : W * (th + 1)], ps)
                    cpi += 1
                # k token ranges: 3x512 + 16 prefix
                for th in range(4):
                    n = W if th < 3 else PRE
                    psf = ps2.tile([P, W], F32, tag="pa", name="pa")
                    ps = psf[:, :n]
                    for db in range(FB):
                        rhs = (
                            xt[db][:, W * th : W * th + n]
                            if th < 3
                            else xt[db][:, KTOK:KTOKP]
                        )
                        nc.tensor.matmul(
                            ps,
                            wk[db][:, P * fb : P * (fb + 1)],
                            rhs,
                            start=(db == 0),
                            stop=(db == FB - 1),
                        )
                    dst = kt[fb][:, W * th : W * th + n] if th < 3 else kt[fb][:, KTOK:KTOKP]
                    copy_engines[cpi % 2](dst, ps)
                    cpi += 1

            # ---- phase A done: free xT and weight-stream pools ----
            phaseA.close()
            phaseB = ExitStack()
            pattn = phaseB.enter_context(tc.tile_pool(name="pattn", bufs=1))
            pwo = phaseB.enter_context(tc.tile_pool(name="pwo", bufs=16))
            pwork = phaseB.enter_context(tc.tile_pool(name="pwork", bufs=2))

            # exp'd-score tiles: one per tail key block + one prefix tile.
            # memset once; each head's exp pass rewrites only the (kb-dependent)
            # valid query range, the rest stays zero forever.
            es = []
            for kb in range(NKB):
                t = pattn.tile([P, W], md, tag=f"es{kb}", name=f"es{kb}")
                msets(t[:], 0.0)
                es.append(t)
            esp = pattn.tile([PRE, W], md, tag="esp", name="esp")
            msets(esp[:], 0.0)

            # ---- attention (out-projection folded in per chunk) ----
            for c in range(NCH):
                ot = [pattn.tile([P, W], md, tag=f"ot{i}", name=f"ot{i}") for i in range(FB)]
                for g in range(FB):  # head pair (2g, 2g+1) on partitions 0/64
                    sc_ps = [[None] * (NKB + 1), [None] * (NKB + 1)]
                    # scores, emitted pairwise for PE row-group concurrency;
                    # moving restricted to the causally-valid query range.
                    for kb in range(NKB + 1):
                        for hr in range(2):
                            r0 = 64 * hr
                            if kb < NKB:
                                qlo = P * max(0, kb - 4)
                                qhi = P * min(3, kb) + P
                                lhsT = kt[g][r0 : r0 + 64, W * c + P * kb : W * c + P * (kb + 1)]
                                ps = ps3.tile([P, W], F32, tag="sc")
                                rhs = qt[g][r0 : r0 + 64, W * c + qlo : W * c + qhi]
                                nc.tensor.matmul(
                                    ps[:, qlo:qhi], lhsT, rhs, start=True, stop=True
                                )
                            else:
                                lhsT = kt[g][r0 : r0 + 64, KTOK:KTOKP]
                                ps = ps3.tile([P, W], F32, tag="sc", name="scp")[:PRE, :]
                                rhs = qt[g][r0 : r0 + 64, W * c : W * (c + 1)]
                                nc.tensor.matmul(ps, lhsT, rhs, start=True, stop=True)
                            sc_ps[hr][kb] = ps
                    # masks (gpsimd: otherwise idle) + exp (scalar)
                    for hr in range(2):
                        for kb in range(NKB):
                            ps = sc_ps[hr][kb]
                            if kb <= 3:
                                qc, moff = P * kb, 0          # strict-upper allow
                            else:
                                qc, moff = P * (kb - 4), P    # lower-incl allow
                            qlo = P * max(0, kb - 4)
                            qhi = P * min(3, kb) + P
                            nc.scalar.activation(
                                es[kb][:, qlo:qhi],
                                ps[:, qlo:qhi],
                                mybir.ActivationFunctionType.Exp,
                                bias=kb_t[:, 9 * c + kb : 9 * c + kb + 1],
                                scale=SCALE,
                            )
                            # causal mask: 0/1 multiply on the exp'd diagonal
                            # block (gpsimd: SBUF-only engine, otherwise idle)
                            nc.gpsimd.tensor_mul(
                                es[kb][:, qc : qc + P],
                                es[kb][:, qc : qc + P],
                                tri_t[:, moff : moff + P],
                            )
                        psp = sc_ps[hr][NKB]
                        nc.scalar.activation(
                            esp[:],
                            psp,
                            mybir.ActivationFunctionType.Exp,
                            bias=kb_t[0:PRE, 9 * c + 8 : 9 * c + 9],
                            scale=SCALE,
                        )
                        if c == 0:
                            # prefix causal mask only matters for global chunk 0,
                            # which is always local chunk 0 (ones elsewhere)
                            nc.gpsimd.tensor_mul(esp[:], esp[:], pm_t[:])
                        # attention * value (+rowsum via ones column)
                        h = 2 * g + hr
                        po = ps2.tile([65, W], F32, tag="av")
                        for kb in range(NKB):
                            nc.tensor.matmul(
                                po,
                                vaug[4 * c + kb][:, 65 * h : 65 * h + 65],
                                es[kb][:],
                                start=(kb == 0),
                                stop=False,
                            )
                        nc.tensor.matmul(
                            po,
                            vaug[NVT][0:PRE, 65 * h : 65 * h + 65],
                            esp[:],
                            start=False,
                            stop=True,
                        )
                        # normalize: recip(rowsum) broadcast via PE, fused into copy
                        rc = pwork.tile([P, W], md, tag="rc", name="rc")
                        nc.vector.reciprocal(rc[64:65, :], po[64:65, :])
                        pb = ps1.tile([64, W], F32, tag="bc")
                        nc.tensor.matmul(
                            pb, ones_t[64:65, 0:64], rc[64:65, :], start=True, stop=True
                        )
                        # hw: a DVE op may read only ONE input from PSUM ->
                        # stage the broadcast tile in SBUF first (scalar: idle)
                        pbs = pwork.tile([64, W], md, tag="pbs", name="pbs")
                        nc.scalar.copy(pbs[:], pb[:])
                        nc.vector.tensor_mul(
                            ot[g][64 * hr : 64 * hr + 64, :],
                            po[0:64, :],
                            pbs[:],
                        )

                # ---- output projection for this chunk ----
                for nh in range(2):
                    wo = []
                    for g in range(FB):
                        t = pwo.tile([P, W], md)
                        nc.sync.dma_start(
                            t[:], woutT[P * g : P * (g + 1), W * nh : W * (nh + 1)]
                        )
                        wo.append(t)
                    for tb in range(W // P):
                        ps = ps2.tile([P, W], F32, tag="pa")
                        for g in range(FB):
                            nc.tensor.matmul(
                                ps,
                                ot[g][:, P * tb : P * (tb + 1)],
                                wo[g][:],
                                start=(g == 0),
                                stop=(g == FB - 1),
                            )
                        ob = pwork.tile([P, W], F32, tag="ob", name="ob")
                        nc.vector.tensor_copy(ob[:], ps)
                        nc.sync.dma_start(
                            out[W * c + P * tb : W * c + P * (tb + 1), W * nh : W * (nh + 1)],
                            ob[:],
                        )
            phaseB.close()

    lp.__exit__(None, None, None)
    nc.compile()
    return nc


def make_core_inputs(x, w_qkv, w_out, mode=MODE):
    """Shard full inputs into per-core input maps (list of 8 dicts)."""
    npdt = ml_dtypes.bfloat16 if mode == "bf16" else np.float32
    x = np.ascontiguousarray(np.asarray(x, dtype=np.float32))
    wqkvT = np.ascontiguousarray(np.asarray(w_qkv, dtype=np.float32).T.astype(npdt))
    woutT = np.ascontiguousarray(np.asarray(w_out, dtype=np.float32).T.astype(npdt))

    tri = np.empty((P, 2 * P), dtype=np.float32)
    t_idx = np.arange(P)[:, None]
    q_idx = np.arange(P)[None, :]
    tri[:, :P] = np.where(t_idx > q_idx, 1.0, 0.0)    # strict-upper allow
    tri[:, P:] = np.where(t_idx <= q_idx, 1.0, 0.0)   # lower-incl allow
    tri = tri.astype(npdt)

    in_maps = []
    for i in range(N_CORES):
        b, j = i // 4, i % 4
        xTl = np.zeros((D, KTOKP), dtype=np.float32)
        r0 = 1024 * j - W
        if j == 0:
            xTl[:, W:KTOK] = x[b, 0 : r0 + KTOK].T
        else:
            xTl[:, :KTOK] = x[b, r0 : r0 + KTOK].T
        xTl[:, KTOK:] = x[b, :PRE].T

        kb = np.zeros((P, NCH * 9), dtype=np.float32)
        pm = np.ones((PRE, W), dtype=np.float32)
        for cl in range(NCH):
            cg = 2 * j + cl
            if cg == 0:
                for kbk in range(4):
                    kb[:, 9 * cl + kbk] = BIAS_NEG
                kb[:PRE, 9 * cl + 4] = BIAS_NEG
                p_i = np.arange(PRE)[:, None]
                q_i = np.arange(W)[None, :]
                pm[:] = np.where(p_i <= q_i, 1.0, 0.0)
            elif cg == 1:
                kb[:PRE, 9 * cl + 0] = BIAS_NEG

        in_maps.append(
            {
                "xT": np.ascontiguousarray(xTl.astype(npdt)),
                "wqkvT": wqkvT,
                "woutT": woutT,
                "trim": tri,
                "kbias": kb,
                "premask": pm.astype(npdt),
            }
        )
    return in_maps


_NC_CACHE = {}


def get_nc(mode=MODE):
    if mode not in _NC_CACHE:
        _NC_CACHE[mode] = build_nc(mode)
    return _NC_CACHE[mode]


def kernel(x, w_qkv, w_out):
    nc = get_nc()
    in_maps = make_core_inputs(x, w_qkv, w_out)
    res = run_bass_kernel_spmd(nc, in_maps, list(range(N_CORES))).results
    out = np.empty((B, S, D), dtype=np.float32)
    for i in range(N_CORES):
        b, j = i // 4, i % 4
        out[b, 1024 * j : 1024 * (j + 1)] = res[i]["out"]
    return out


# revision 11
# speedup vs baseline: 1.0896x; 1.0896x over previous
"""Trainium2 Bass kernel for chunked sliding-window + global-prefix causal self-attention.

Module: x[B,S,D] -> qkv proj -> windowed attention (W=512 window, 16 global
prefix keys) -> out proj.  B=2, S=4096, D=1024, H=16, Dh=64.

Sharding (8 cores): sequence-parallel over query chunk pairs.
  core i: batch b = i//4, query chunks (2j, 2j+1), j = i%4  -> 1024 query rows.
  Each core re-projects the K/V rows its chunks attend to (3 chunks of 512 =
  1536 kv tokens, front chunk zero-padded for j=0) plus the 16 prefix tokens.
  Output is row-sharded -> gather is pure concatenation.

On-chip layout is feature-major (xT = x.T fed from host), so no transposes
are ever needed:
  qT/kT  [f, tok]   (w stationary, xT moving)
  V      [tok, f]   (xT stationary, w moving), augmented with a ones column
                    per head so the attention-value matmul also produces
                    softmax row-sums for free.
  scores are computed transposed S.T [key, query]; the score matmul moving
  range is restricted to the causally-valid query window per key block.
  Row-sums for all 16 heads of a chunk are DMA-collected into one [16,W]
  tile so the softmax reciprocal is a single full-width DVE op; the
  normalization multiply is applied in place on the o.T tiles before the
  output projection.
"""

import os
import sys

import numpy as np

for _p in ("/opt/trn_rl_repo", os.path.expanduser("~/trn_rl_repo")):
    if os.path.isdir(_p) and _p not in sys.path:
        sys.path.insert(0, _p)

import ml_dtypes  # noqa: E402

import concourse.bass as bass  # noqa: E402
import concourse.mybir as mybir  # noqa: E402
import concourse.tile as tile  # noqa: E402
from concourse import bacc  # noqa: E402
from concourse.bass_utils import run_bass_kernel_spmd  # noqa: E402

F32 = mybir.dt.float32
F32R = mybir.dt.float32r
BF16 = mybir.dt.bfloat16

# problem constants (hardcoded per contest contract)
B, S, D, H, DH = 2, 4096, 1024, 16, 64
W, PRE = 512, 16
P = 128
NCH = 2                  # query chunks per core
TOK = NCH * W            # 1024 query tokens per core
KTOK = TOK + W           # 1536 kv tokens per core
KTOKP = KTOK + PRE       # + prefix tokens appended as cols [1536:1552]
FB = D // P              # 8 feature blocks of 128
NKB = 8                  # tail key blocks (128 keys) per chunk
NVT = KTOK // P          # 12 tail token tiles; tile 12 = prefix tokens
SCALE = 1.0 / 8.0        # 1/sqrt(DH)
NEG = -10000.0           # additive mask value (pre-scale)
BIAS_NEG = -50.0         # ACT bias mask value (post-scale)
N_CORES = 8

MODE = os.environ.get("KERNEL_MODE", "bf16")  # bf16 | f32r | f32


def build_nc(mode=MODE):
    nc = bacc.Bacc("TRN2", target_bir_lowering=False, debug=False)
    md = {"bf16": BF16, "f32r": F32R, "f32": F32}[mode]
    is4 = md != BF16  # 4-byte storage dtypes share fp32 bit layout

    xT = nc.dram_tensor("xT", [D, KTOKP], md, kind="ExternalInput").ap()
    wqkvT = nc.dram_tensor("wqkvT", [D, 3 * D], md, kind="ExternalInput").ap()
    woutT = nc.dram_tensor("woutT", [D, D], md, kind="ExternalInput").ap()
    trim = nc.dram_tensor("trim", [P, 2 * P], md, kind="ExternalInput").ap()
    kbias = nc.dram_tensor("kbias", [P, NCH * 9], F32, kind="ExternalInput").ap()
    premask = nc.dram_tensor("premask", [PRE, W], md, kind="ExternalInput").ap()
    out = nc.dram_tensor("out", [TOK, D], F32, kind="ExternalOutput").ap()

    from contextlib import ExitStack

    def msets(t, val):
        nc.gpsimd.memset(t.bitcast(F32) if is4 else t, val)

    lp = nc.allow_low_precision(reason="bf16/fp32r matmul chain within 2e-2 rel tol")
    lp.__enter__()
    with tile.TileContext(nc) as tc:
        with (
            tc.tile_pool(name="pp", bufs=1) as pp,                    # persistent tiles
            tc.tile_pool(name="ps2", bufs=2, space="PSUM") as ps2,    # proj / av
            tc.tile_pool(name="ps3", bufs=3, space="PSUM") as ps3,    # scores
            tc.tile_pool(name="ps1", bufs=1, space="PSUM") as ps1,    # bcast
            tc.tile_pool(name="pes", bufs=1) as pes,                  # es double-buffer
        ):
            phaseA = ExitStack()
            # exp'd-score tiles, double-buffered by (g,hr) parity so the exp
            # for the next head can overwrite while the AV chain of the
            # current head still streams the other slot.  memset once, here,
            # so the gpsimd work overlaps phase A and the PE never stalls at
            # the phase boundary; each head's exp pass rewrites only the
            # (kb-dependent) valid query range, the rest stays zero forever.
            es2 = []
            esp2 = []
            for sl in range(2):
                row = []
                for kb in range(NKB):
                    t = pes.tile([P, W], md, tag=f"es{sl}_{kb}", name=f"es{sl}_{kb}")
                    msets(t[:], 0.0)
                    row.append(t)
                es2.append(row)
                t = pes.tile([PRE, W], md, tag=f"esp{sl}", name=f"esp{sl}")
                msets(t[:], 0.0)
                esp2.append(t)
            px = phaseA.enter_context(tc.tile_pool(name="px", bufs=1))
            pwqk = phaseA.enter_context(tc.tile_pool(name="pwqk", bufs=16))
            pwv = phaseA.enter_context(tc.tile_pool(name="pwv", bufs=8))
            # ---- constants / masks ----
            tri_t = pp.tile([P, 2 * P], md, tag="tri")
            nc.sync.dma_start(tri_t[:], trim)
            kb_t = pp.tile([P, NCH * 9], F32, tag="kbias")
            nc.sync.dma_start(kb_t[:], kbias)
            pm_t = pp.tile([PRE, W], md, tag="premask")
            nc.sync.dma_start(pm_t[:], premask)
            ones_t = pp.tile([P, 64], md, tag="ones")
            msets(ones_t[:], 1.0)

            # ---- load xT (feature-major activations) ----
            xt = []
            for i in range(FB):
                t = px.tile([P, KTOKP], md, tag=f"xt{i}")
                nc.sync.dma_start(t[:], xT[P * i : P * (i + 1), :])
                xt.append(t)

            # ---- V projection: V[tok, f] (+ per-head ones column) ----
            # vaug[tb] cols: head h -> [65h : 65h+64] = V, col 65h+64 = 1.0
            vaug = []
            for tb in range(NVT + 1):
                t = pp.tile([P, H * 65], md, tag=f"va{tb}")
                msets(t.rearrange("p (h e) -> p h e", e=65)[:, :, 64:65], 1.0)
                vaug.append(t)
            for vh in range(2):  # 512-wide halves of the 1024 v features
                wv = []
                for db in range(FB):
                    t = pwv.tile([P, W], md)
                    nc.sync.dma_start(
                        t[:], wqkvT[P * db : P * (db + 1), 2 * D + W * vh : 2 * D + W * (vh + 1)]
                    )
                    wv.append(t)
                for tb in range(NVT + 1):
                    ps = ps2.tile([P, W], F32, tag="pa")
                    ncols = P if tb < NVT else PRE
                    for db in range(FB):
                        nc.tensor.matmul(
                            ps[:ncols, :],
                            xt[db][:, P * tb : P * tb + ncols],
                            wv[db][:],
                            start=(db == 0),
                            stop=(db == FB - 1),
                        )
                    # scatter into vaug stride-65 head slots (scalar engine: idle here)
                    nc.scalar.copy(
                        vaug[tb].rearrange("p (h e) -> p h e", e=65)[
                            :ncols, 8 * vh : 8 * (vh + 1), 0:64
                        ],
                        ps.rearrange("p (h e) -> p h e", e=64)[:ncols, 0:8, :],
                    )

            # ---- Q/K projections: qT/kT [f, tok] ----
            # whole-row weight slabs (contiguous 2KB DMA lines), V-proj style
            # uninterrupted 8-matmul accumulation chains per output tile.
            qt = [pp.tile([P, TOK], md, tag=f"qt{i}", name=f"qt{i}") for i in range(FB)]
            kt = [pp.tile([P, KTOKP], md, tag=f"kt{i}", name=f"kt{i}") for i in range(FB)]
            wq = []
            wk = []
            for db in range(FB):
                t = pwqk.tile([P, D], md)
                nc.sync.dma_start(t[:], wqkvT[P * db : P * (db + 1), 0:D])
                wq.append(t)
            for db in range(FB):
                t = pwqk.tile([P, D], md)
                nc.sync.dma_start(t[:], wqkvT[P * db : P * (db + 1), D : 2 * D])
                wk.append(t)
            copy_engines = [nc.vector.tensor_copy, nc.scalar.copy]
            cpi = 0
            for fb in range(FB):
                # q token halves {0,1} (query tokens = xT cols [512:1536))
                for th in range(2):
                    ps = ps2.tile([P, W], F32, tag="pa")
                    for db in range(FB):
                        nc.tensor.matmul(
                            ps,
                            wq[db][:, P * fb : P * (fb + 1)],
                            xt[db][:, W + W * th : W + W * (th + 1)],
                            start=(db == 0),
                            stop=(db == FB - 1),
                        )
                    copy_engines[cpi % 2](qt[fb][:, W * th : W * (th + 1)], ps)
                    cpi += 1
                # k token ranges: 3x512 + 16 prefix
                for th in range(4):
                    n = W if th < 3 else PRE
                    psf = ps2.tile([P, W], F32, tag="pa", name="pa")
                    ps = psf[:, :n]
                    for db in range(FB):
                        rhs = (
                            xt[db][:, W * th : W * th + n]
                            if th < 3
                            else xt[db][:, KTOK:KTOKP]
                        )
                        nc.tensor.matmul(
                            ps,
                            wk[db][:, P * fb : P * (fb + 1)],
                            rhs,
                            start=(db == 0),
                            stop=(db == FB - 1),
                        )
                    dst = kt[fb][:, W * th : W * th + n] if th < 3 else kt[fb][:, KTOK:KTOKP]
                    copy_engines[cpi % 2](dst, ps)
                    cpi += 1

            # ---- phase A done: free xT and weight-stream pools ----
            phaseA.close()
            phaseB = ExitStack()
            pattn = phaseB.enter_context(tc.tile_pool(name="pattn", bufs=1))
            pwo = phaseB.enter_context(tc.tile_pool(name="pwo", bufs=16))
            pwork = phaseB.enter_context(tc.tile_pool(name="pwork", bufs=2))

            # ---- attention (out-projection folded in per chunk) ----
            for c in range(NCH):
                ot = [pattn.tile([P, W], md, tag=f"ot{i}", name=f"ot{i}") for i in range(FB)]
                for g in range(FB):  # head pair (2g, 2g+1) on partitions 0/64
                    sc_ps = [[None] * (NKB + 1), [None] * (NKB + 1)]
                    # scores, emitted pairwise for PE row-group concurrency;
                    # moving restricted to the causally-valid query range.
                    for kb in range(NKB + 1):
                        for hr in range(2):
                            r0 = 64 * hr
                            if kb < NKB:
                                qlo = P * max(0, kb - 4)
                                qhi = P * min(3, kb) + P
                                lhsT = kt[g][r0 : r0 + 64, W * c + P * kb : W * c + P * (kb + 1)]
                                ps = ps3.tile([P, W], F32, tag="sc")
                                rhs = qt[g][r0 : r0 + 64, W * c + qlo : W * c + qhi]
                                nc.tensor.matmul(
                                    ps[:, qlo:qhi], lhsT, rhs, start=True, stop=True
                                )
                            else:
                                lhsT = kt[g][r0 : r0 + 64, KTOK:KTOKP]
                                ps = ps3.tile([P, W], F32, tag="sc", name="scp")[:PRE, :]
                                rhs = qt[g][r0 : r0 + 64, W * c : W * (c + 1)]
                                nc.tensor.matmul(ps, lhsT, rhs, start=True, stop=True)
                            sc_ps[hr][kb] = ps
                    # masks (gpsimd: otherwise idle) + exp (scalar)
                    for hr in range(2):
                        sl = (2 * g + hr) % 2
                        es = es2[sl]
                        esp = esp2[sl]
                        for kb in range(NKB):
                            ps = sc_ps[hr][kb]
                            if kb <= 3:
                                qc, moff = P * kb, 0          # strict-upper allow
                            else:
                                qc, moff = P * (kb - 4), P    # lower-incl allow
                            qlo = P * max(0, kb - 4)
                            qhi = P * min(3, kb) + P
                            nc.scalar.activation(
                                es[kb][:, qlo:qhi],
                                ps[:, qlo:qhi],
                                mybir.ActivationFunctionType.Exp,
                                bias=kb_t[:, 9 * c + kb : 9 * c + kb + 1],
                                scale=SCALE,
                            )
                            # causal mask: 0/1 multiply on the exp'd diagonal
                            # block (gpsimd: SBUF-only engine, otherwise idle)
                            nc.gpsimd.tensor_mul(
                                es[kb][:, qc : qc + P],
                                es[kb][:, qc : qc + P],
                                tri_t[:, moff : moff + P],
                            )
                        psp = sc_ps[hr][NKB]
                        nc.scalar.activation(
                            esp[:],
                            psp,
                            mybir.ActivationFunctionType.Exp,
                            bias=kb_t[0:PRE, 9 * c + 8 : 9 * c + 9],
                            scale=SCALE,
                        )
                        if c == 0:
                            # prefix causal mask only matters for global chunk 0,
                            # which is always local chunk 0 (ones elsewhere)
                            nc.gpsimd.tensor_mul(esp[:], esp[:], pm_t[:])
                        # attention * value (+rowsum via ones column)
                        h = 2 * g + hr
                        po = ps2.tile([65, W], F32, tag="av")
                        for kb in range(NKB):
                            nc.tensor.matmul(
                                po,
                                vaug[4 * c + kb][:, 65 * h : 65 * h + 65],
                                es[kb][:],
                                start=(kb == 0),
                                stop=False,
                            )
                        nc.tensor.matmul(
                            po,
                            vaug[NVT][0:PRE, 65 * h : 65 * h + 65],
                            esp[:],
                            start=False,
                            stop=True,
                        )
                        # normalize: recip(rowsum) broadcast via PE, fused into copy
                        rc = pwork.tile([P, W], md, tag="rc", name="rc")
                        nc.vector.reciprocal(rc[64:65, :], po[64:65, :])
                        pb = ps1.tile([64, W], F32, tag="bc")
                        nc.tensor.matmul(
                            pb, ones_t[64:65, 0:64], rc[64:65, :], start=True, stop=True
                        )
                        # hw: a DVE op may read only ONE input from PSUM ->
                        # stage the broadcast tile in SBUF first (scalar: idle)
                        pbs = pwork.tile([64, W], md, tag="pbs", name="pbs")
                        nc.scalar.copy(pbs[:], pb[:])
                        nc.vector.tensor_mul(
                            ot[g][64 * hr : 64 * hr + 64, :],
                            po[0:64, :],
                            pbs[:],
                        )

                # ---- output projection for this chunk ----
                for nh in range(2):
                    wo = []
                    for g in range(FB):
                        t = pwo.tile([P, W], md)
                        nc.sync.dma_start(
                            t[:], woutT[P * g : P * (g + 1), W * nh : W * (nh + 1)]
                        )
                        wo.append(t)
                    for tb in range(W // P):
                        ps = ps2.tile([P, W], F32, tag="pa")
                        for g in range(FB):
                            nc.tensor.matmul(
                                ps,
                                ot[g][:, P * tb : P * (tb + 1)],
                                wo[g][:],
                                start=(g == 0),
                                stop=(g == FB - 1),
                            )
                        ob = pwork.tile([P, W], F32, tag="ob", name="ob")
                        nc.vector.tensor_copy(ob[:], ps)
                        nc.sync.dma_start(
                            out[W * c + P * tb : W * c + P * (tb + 1), W * nh : W * (nh + 1)],
                            ob[:],
                        )
            phaseB.close()

    lp.__exit__(None, None, None)
    nc.compile()
    return nc


def make_core_inputs(x, w_qkv, w_out, mode=MODE):
    """Shard full inputs into per-core input maps (list of 8 dicts)."""
    npdt = ml_dtypes.bfloat16 if mode == "bf16" else np.float32
    x = np.ascontiguousarray(np.asarray(x, dtype=np.float32))
    wqkvT = np.ascontiguousarray(np.asarray(w_qkv, dtype=np.float32).T.astype(npdt))
    woutT = np.ascontiguousarray(np.asarray(w_out, dtype=np.float32).T.astype(npdt))

    tri = np.empty((P, 2 * P), dtype=np.float32)
    t_idx = np.arange(P)[:, None]
    q_idx = np.arange(P)[None, :]
    tri[:, :P] = np.where(t_idx > q_idx, 1.0, 0.0)    # strict-upper allow
    tri[:, P:] = np.where(t_idx <= q_idx, 1.0, 0.0)   # lower-incl allow
    tri = tri.astype(npdt)

    in_maps = []
    for i in range(N_CORES):
        b, j = i // 4, i % 4
        xTl = np.zeros((D, KTOKP), dtype=np.float32)
        r0 = 1024 * j - W
        if j == 0:
            xTl[:, W:KTOK] = x[b, 0 : r0 + KTOK].T
        else:
            xTl[:, :KTOK] = x[b, r0 : r0 + KTOK].T
        xTl[:, KTOK:] = x[b, :PRE].T

        kb = np.zeros((P, NCH * 9), dtype=np.float32)
        pm = np.ones((PRE, W), dtype=np.float32)
        for cl in range(NCH):
            cg = 2 * j + cl
            if cg == 0:
                for kbk in range(4):
                    kb[:, 9 * cl + kbk] = BIAS_NEG
                kb[:PRE, 9 * cl + 4] = BIAS_NEG
                p_i = np.arange(PRE)[:, None]
                q_i = np.arange(W)[None, :]
                pm[:] = np.where(p_i <= q_i, 1.0, 0.0)
            elif cg == 1:
                kb[:PRE, 9 * cl + 0] = BIAS_NEG

        in_maps.append(
            {
                "xT": np.ascontiguousarray(xTl.astype(npdt)),
                "wqkvT": wqkvT,
                "woutT": woutT,
                "trim": tri,
                "kbias": kb,
                "premask": pm.astype(npdt),
            }
        )
    return in_maps


_NC_CACHE = {}


def get_nc(mode=MODE):
    if mode not in _NC_CACHE:
        _NC_CACHE[mode] = build_nc(mode)
    return _NC_CACHE[mode]


def kernel(x, w_qkv, w_out):
    nc = get_nc()
    in_maps = make_core_inputs(x, w_qkv, w_out)
    res = run_bass_kernel_spmd(nc, in_maps, list(range(N_CORES))).results
    out = np.empty((B, S, D), dtype=np.float32)
    for i in range(N_CORES):
        b, j = i // 4, i % 4
        out[b, 1024 * j : 1024 * (j + 1)] = res[i]["out"]
    return out
